# revision 26
# baseline (speedup 1.0000x reference)
"""GAT (3-layer, 4-head) forward on 8 Trainium2 NeuronCores.

Strategy: nodes are partitioned across the 8 cores (destination-sharded);
each core processes the in-edges of its nodes in a CSR layout
[128 dst nodes (partitions) x W in-edge slots (free dim)], gathering source
node features with the Q7 dma_gather instruction from a replicated node
feature table. Self-loops are folded into the CSR as ordinary edges. Pad
slots point at a dedicated pad table row whose a_src is -1e30 so exp -> 0
(no mask buffers). Per-layer node features (h | a_src | a_dst) are produced
by a sharded dense matmul and exchanged with sliced AllGather collectives
that overlap the block loop. Score chain (leaky-relu + exp) runs on the
Scalar engine with a_dst as the activation bias; messages are weighted and
fold-reduced on the Vector engine in bf16.

Self-contained: builds/compiles the Bass program on first call from the
actual inputs, runs SPMD on cores 0-7, reassembles the full output.
"""

import sys

for _p in ("/opt/trn_rl_repo",):
    if _p not in sys.path:
        sys.path.insert(0, _p)

import numpy as np

import concourse.bass as bass
import concourse.mybir as mybir
import concourse.tile as tile
from concourse import bacc, bass_utils

F32 = mybir.dt.float32
BF = mybir.dt.bfloat16
I16 = mybir.dt.int16
AX = mybir.AluOpType
AF = mybir.ActivationFunctionType

NC = 8          # cores
P = 128         # partitions / block size
H, C = 4, 64    # heads, channels
HC = H * C      # 256
EXT = HC + 2 * H          # 264 = h | a_src | a_dst (f32 logical)
EXTB = HC + 4 * H         # 272 bf16 elems: 256 bf16 h + 16 slots holding 8 f32 scores
ROW = 384                 # table row stride in bf16 elems (768B, mult of 256B)
WCAP = 8                  # max gather part width (edge slots per partition)
NEG_SLOPE = 0.2
NSLICE = 4                # AllGather slices per layer

_cache = {}


def _build_wext(w, att_src, att_dst):
    # h = x@w ; a_src[n,h] = sum_c h[n,h*C+c]*att_src[h,c]  ->  x @ (w @ M)
    m_src = np.zeros((HC, H), np.float32)
    m_dst = np.zeros((HC, H), np.float32)
    for hh in range(H):
        m_src[hh * C:(hh + 1) * C, hh] = att_src[hh]
        m_dst[hh * C:(hh + 1) * C, hh] = att_dst[hh]
    return np.concatenate([w, w @ m_src, w @ m_dst], axis=1).astype(np.float32)  # [din, 264]


def _host_prep(x, edge_index, params):
    N = x.shape[0]
    IN = x.shape[1]
    src0 = np.asarray(edge_index[0], np.int64).astype(np.int32)
    dst0 = np.asarray(edge_index[1], np.int64).astype(np.int32)
    # fold self-loops into the edge list (PyG GATConv adds them)
    loop = np.arange(N, dtype=np.int32)
    src = np.concatenate([src0, loop])
    dst = np.concatenate([dst0, loop])

    half_id = N // 2                       # node-id split for lo/hi tables
    # every core owns nodes from BOTH halves: KBH lo-blocks then KBH hi-blocks.
    # The lo/hi tables are then separate Shared tensors each fed by ONE
    # AllGather, and the lo AG overlaps the second half of the block loop.
    KBH = -(-(-(-(N - half_id) // P)) // NC)    # per-core blocks per half
    KB = 2 * KBH
    CH_CAP = KB * P                        # rows per core in table
    HALFT = NC * KBH * P                   # rows in each half table
    TAB = 2 * HALFT

    lo_deg = np.bincount(dst[src < half_id], minlength=N)
    hi_deg = np.bincount(dst[src >= half_id], minlength=N)

    # per id-half, z-order nodes by (lo,hi) degree so both per-block maxima
    # stay tight; deal blocks of 128 round-robin across ALL 8 cores.
    core_nodes = [[] for _ in range(NC)]   # per core: node ids in block/slot order (-1 pad)
    for half in range(2):
        ids = np.arange(half_id) if half == 0 else np.arange(half_id, N)
        l_, h_ = lo_deg[ids], hi_deg[ids]
        a = np.clip(63 - l_, 0, 63).astype(np.int64)
        b = np.clip(63 - h_, 0, 63).astype(np.int64)
        z = np.zeros_like(a)
        for i in range(6):
            z |= ((a >> i) & 1) << (2 * i)
            z |= ((b >> i) & 1) << (2 * i + 1)
        order = ids[np.argsort(z, kind="stable")]
        padded = np.full(NC * KBH * P, -1, np.int64)
        padded[:order.size] = order
        blocks = padded.reshape(-1, P)      # [NC*KBH, 128] global sorted blocks
        for g in range(blocks.shape[0]):
            c = g % NC
            core_nodes[c].append(blocks[g])
    core_nodes = [np.concatenate(b) for b in core_nodes]   # [CH_CAP] node ids (-1 pad)

    # table position of every node: lo rows [0, HALFT), hi rows [HALFT, TAB).
    # (c, k, slot) -> half*HALFT + c*KBH*P + (k % KBH)*P + slot
    tab_pos = np.zeros(N, np.int64)
    for c in range(NC):
        blk = core_nodes[c].reshape(KB, P)
        for k in range(KB):
            half = k // KBH
            rows = half * HALFT + c * KBH * P + (k % KBH) * P
            b = blk[k]
            real = b >= 0
            tab_pos[b[real]] = rows + np.nonzero(real)[0]

    # guaranteed pad rows in each half table (a_src forced to -1e30 there)
    pad_lo = pad_hi = -1
    for c in range(NC):
        blk = core_nodes[c].reshape(KB, P)
        for k in range(KB):
            padpos = np.nonzero(blk[k] < 0)[0]
            if padpos.size:
                half = k // KBH
                row = c * KBH * P + (k % KBH) * P + int(padpos[0])
                if half == 0 and pad_lo < 0:
                    pad_lo = row
                if half == 1 and pad_hi < 0:
                    pad_hi = row
    assert pad_lo >= 0 and pad_hi >= 0

    # W schedule per k (uniform over all cores): max lo/hi degree in any core's k-th block
    Wlo = np.zeros(KB, np.int64)
    Whi = np.zeros(KB, np.int64)
    for c in range(NC):
        blk = core_nodes[c].reshape(KB, P)
        for k in range(KB):
            real = blk[k][blk[k] >= 0]
            if real.size:
                Wlo[k] = max(Wlo[k], lo_deg[real].max())
                Whi[k] = max(Whi[k], hi_deg[real].max())

    # part schedule (same for every core): per k, parts of width <= WCAP
    # (k, tbl, w, col_in_block, idxcol)
    parts = []
    icol = 0
    for k in range(KB):
        col = 0
        for tbl, Wt in ((0, int(Wlo[k])), (1, int(Whi[k]))):
            rem = Wt
            while rem > 0:
                w = min(rem, WCAP)
                parts.append((k, tbl, w, col, icol))
                col += w
                icol += 8 * w
                rem -= w
    tot_icol = max(icol, 8)

    # edge lists grouped by dst
    order_e = np.argsort(dst, kind="stable")
    src_s = src[order_e]
    dst_s = dst[order_e]
    seg_start = np.searchsorted(dst_s, np.arange(N))
    seg_end = np.searchsorted(dst_s, np.arange(N) + 1)

    idx_bufs, xtt_bufs, smk_bufs = [], [], []
    for core in range(NC):
        blk = core_nodes[core].reshape(KB, P)
        sumW = int((Wlo + Whi).sum())
        kcol0 = np.concatenate([[0], np.cumsum(Wlo + Whi)])
        idxmat = np.zeros((P, sumW), np.int32)
        # default pad targets per column (lo cols -> pad_lo, hi cols -> pad_hi)
        for k in range(KB):
            b = int(kcol0[k])
            idxmat[:, b:b + int(Wlo[k])] = pad_lo
            idxmat[:, b + int(Wlo[k]):b + int(Wlo[k]) + int(Whi[k])] = pad_hi
        for k in range(KB):
            base = int(kcol0[k])
            for s in range(P):
                n = blk[k, s]
                if n < 0:
                    continue
                es, ee = seg_start[n], seg_end[n]
                nbrs = tab_pos[src_s[es:ee]]
                nlo = nbrs[nbrs < HALFT]
                nhi = nbrs[nbrs >= HALFT] - HALFT
                idxmat[s, base:base + nlo.size] = nlo
                hb = base + int(Wlo[k])
                idxmat[s, hb:hb + nhi.size] = nhi
        # wrapped+replicated idx buffer per part
        idx_buf = np.zeros((P, tot_icol), np.int16)
        for (k, tbl, w, col, ic) in parts:
            base = int(kcol0[k]) + col
            sl = idxmat[:, base:base + w]          # [128, w]
            vals = sl.T.reshape(-1)                # flat i = j*128+p
            NI = P * w
            wrapped = vals.reshape(NI // 16, 16).T.astype(np.int16)   # [16, NI/16]
            idx_buf[:, ic:ic + 8 * w] = np.tile(wrapped, (8, 1))
        idx_bufs.append(idx_buf)

        # xT tiles [KB, 64, 128]
        KIN = 64
        xtt = np.zeros((KB, KIN, P), np.float32)
        for k in range(KB):
            for s in range(P):
                n = blk[k, s]
                if n >= 0:
                    xtt[k, :IN, s] = x[n]
        xtt_bufs.append(xtt)

        smk = np.zeros((KB, P, 1), np.float32)
        smk[blk.reshape(KB, P, 1) < 0] = -1e30
        smk_bufs.append(smk)

    consts = {}
    w0e = _build_wext(params["w0"], params["att_src0"], params["att_dst0"])
    w0p = np.zeros((64, EXT), np.float32)
    w0p[:IN] = w0e
    consts["w0ext"] = w0p
    for l in (1, 2):
        we = _build_wext(params[f"w{l}"], params[f"att_src{l}"], params[f"att_dst{l}"])
        consts[f"w{l}ext"] = we.reshape(2, P, EXT).copy()
    consts["bias"] = np.stack([np.tile(params[f"b{l}"][None, :], (P, 1)) for l in range(3)])
    consts["identity"] = np.eye(P, dtype=np.float32)
    consts["hw1"] = np.asarray(params["head_w1"], np.float32).reshape(2, P, C)
    consts["hb1"] = np.tile(np.asarray(params["head_b1"], np.float32)[None, :], (P, 1))
    consts["hw2"] = np.asarray(params["head_w2"], np.float32).reshape(C, 1)
    hb2 = float(np.asarray(params["head_b2"]).reshape(-1)[0])

    geom = dict(N=N, E=src.shape[0], KB=KB, KBH=KBH, CH_CAP=CH_CAP, TAB=TAB,
                HALFT=HALFT, tot_icol=tot_icol, hb2=hb2, parts=parts,
                Wlo=Wlo, Whi=Whi)
    return geom, idx_bufs, xtt_bufs, smk_bufs, consts, core_nodes


def _build_program(geom, consts):
    KB = geom["KB"]
    KBH = geom["KBH"]
    CH_CAP = geom["CH_CAP"]
    TAB = geom["TAB"]
    HALFT = geom["HALFT"]
    parts = geom["parts"]
    hb2 = geom["hb2"]
    Wlo, Whi = geom["Wlo"], geom["Whi"]
    WT = [int(Wlo[k] + Whi[k]) for k in range(KB)]
    WTmax = max(WT)

    # AllGather slice boundaries (in blocks)
    sb = KB // NSLICE
    slice_end = [min((s + 1) * sb + (KB % NSLICE if s == NSLICE - 1 else 0), KB)
                 for s in range(NSLICE)]
    slice_end[-1] = KB
    slice_start = [0] + slice_end[:-1]

    nc = bacc.Bacc("TRN2", target_bir_lowering=False, debug=False,
                   num_devices=NC, num_swdge_queues=4)

    xtt_d = nc.dram_tensor("xtt", [KB, 64, P], F32, kind="ExternalInput")
    idx_d = nc.dram_tensor("idxbuf", [P, geom["tot_icol"]], I16, kind="ExternalInput")
    smk_d = nc.dram_tensor("srcmask", [KB, P, 1], F32, kind="ExternalInput")
    w0e_d = nc.dram_tensor("w0ext", [64, EXT], F32, kind="ExternalInput")
    w1e_d = nc.dram_tensor("w1ext", [2, P, EXT], F32, kind="ExternalInput")
    w2e_d = nc.dram_tensor("w2ext", [2, P, EXT], F32, kind="ExternalInput")
    bias_d = nc.dram_tensor("bias", [3, P, HC], F32, kind="ExternalInput")
    iden_d = nc.dram_tensor("identity", [P, P], F32, kind="ExternalInput")
    hw1_d = nc.dram_tensor("hw1", [2, P, C], F32, kind="ExternalInput")
    hb1_d = nc.dram_tensor("hb1", [P, C], F32, kind="ExternalInput")
    hw2_d = nc.dram_tensor("hw2", [C, 1], F32, kind="ExternalInput")
    out_d = nc.dram_tensor("outv", [CH_CAP, 1], F32, kind="ExternalOutput")

    qrr = [0]

    def next_q():
        q = qrr[0]
        qrr[0] = (q + 1) % 4
        return q

    with tile.TileContext(nc) as tc:
        with (
            tc.tile_pool(name="dram", bufs=1, space="DRAM") as dram,
            tc.tile_pool(name="consts", bufs=1) as cpool,
            tc.tile_pool(name="gp", bufs=10) as gp,
            tc.tile_pool(name="swp", bufs=3) as swp,
            tc.tile_pool(name="prodp", bufs=2) as prodp,
            tc.tile_pool(name="sp", bufs=6) as spool,
            tc.tile_pool(name="psum", bufs=2, space="PSUM") as pp,
            tc.tile_pool(name="psum2", bufs=2, space="PSUM") as pp2,
        ):
            bounce = [dram.tile([CH_CAP, ROW], BF, name=f"bounce{l}", tag=f"bounce{l}")
                      for l in range(3)]
            tabsLO = [dram.tile([HALFT, ROW], BF, name=f"tabL{l}", tag=f"tabL{l}",
                                addr_space="Shared") for l in range(3)]
            tabsHI = [dram.tile([HALFT, ROW], BF, name=f"tabH{l}", tag=f"tabH{l}",
                                addr_space="Shared") for l in range(3)]

            w0e = cpool.tile([64, EXT], F32, name="w0e")
            nc.sync.dma_start(w0e[:], w0e_d[:])
            w1e = cpool.tile([P, 2, EXT], F32, name="w1e")
            nc.sync.dma_start(w1e[:], w1e_d[:].rearrange("a p e -> p a e"))
            w2e = cpool.tile([P, 2, EXT], F32, name="w2e")
            nc.sync.dma_start(w2e[:], w2e_d[:].rearrange("a p e -> p a e"))
            bias = cpool.tile([P, 3, HC], F32, name="bias")
            nc.sync.dma_start(bias[:], bias_d[:].rearrange("a p e -> p a e"))
            iden = cpool.tile([P, P], F32, name="iden")
            nc.sync.dma_start(iden[:], iden_d[:])
            hw1 = cpool.tile([P, 2, C], F32, name="hw1")
            nc.sync.dma_start(hw1[:], hw1_d[:].rearrange("a p e -> p a e"))
            hb1 = cpool.tile([P, C], F32, name="hb1")
            nc.sync.dma_start(hb1[:], hb1_d[:])
            hw2 = cpool.tile([C, 1], F32, name="hw2")
            nc.sync.dma_start(hw2[:], hw2_d[:])
            smk = cpool.tile([P, KB], F32, name="smk")
            nc.sync.dma_start(smk[:], smk_d[:].rearrange("k p o -> p (k o)"))
            itb = cpool.tile([P, geom["tot_icol"]], I16, name="itb")
            nc.sync.dma_start(itb[:], idx_d[:])
            zero1 = cpool.tile([P, 1], F32, name="zero1")
            nc.vector.memset(zero1[:], 0.0)
            adst = cpool.tile([P, KB, H], F32, name="adst")

            def write_hb(ps, k, dst_dram_rows, l):
                """Copy matmul output (PSUM, [P, EXT] f32) to bf16 table-row
                layout and DMA to bounce; record a_dst; apply pad mask."""
                hbt = spool.tile([P, EXTB], BF, name="hb", tag="hb")
                sc = hbt[:, HC:EXTB].bitcast(F32)          # [P, 8] f32
                nc.vector.tensor_copy(out=hbt[:, 0:HC], in_=ps[:, 0:HC])
                nc.vector.tensor_tensor(out=sc[:, 0:H], in0=ps[:, HC:HC + H],
                                        in1=smk[:, k:k + 1].to_broadcast([P, H]),
                                        op=AX.add)
                nc.vector.tensor_copy(out=sc[:, H:2 * H], in_=ps[:, HC + H:EXT])
                nc.vector.tensor_copy(out=adst[:, k, :], in_=ps[:, HC + H:EXT])
                nc.sync.dma_start(dst_dram_rows, hbt[:])

            def ag_half(l, half):
                r0, r1 = half * KBH * P, (half + 1) * KBH * P
                outt = tabsLO[l] if half == 0 else tabsHI[l]
                nc.gpsimd.collective_compute(
                    "AllGather", AX.bypass, replica_groups=[list(range(NC))],
                    ins=[bounce[l][r0:r1, :]], outs=[outt.opt()])

            # ---- layer-0 dense phase: h0 = x @ W0ext (sharded: own nodes only)
            for k in range(KB):
                xt = spool.tile([64, P], F32, name="xt", tag="xt")
                nc.sync.dma_start(xt[:], xtt_d[k])
                ps = pp.tile([P, EXT], F32, name="psmm", tag="psmm")
                nc.tensor.matmul(ps[:], lhsT=xt[:], rhs=w0e[:], start=True, stop=True)
                write_hb(ps, k, bounce[0][k * P:(k + 1) * P, 0:EXTB], 0)
                if k == KBH - 1:
                    ag_half(0, 0)
            ag_half(0, 1)

            # ---- 3 GAT layers
            for l in range(3):
                for k in range(KB):
                    blk_parts = [pt for pt in parts if pt[0] == k]
                    Wt = WT[k]
                    sw = swp.tile([P, WTmax, H], F32, name="sw", tag="sw")
                    sb2 = swp.tile([P, WTmax, H], F32, name="sb2", tag="sb2")
                    swb = swp.tile([P, WTmax, H], BF, name="swb", tag="swb")
                    prod = prodp.tile([P, WTmax, HC], BF, name="prod", tag="prod")
                    gts = []
                    for (kk, tbl, w, col, ic) in blk_parts:
                        g = gp.tile([P, WCAP, ROW], BF, name="g", tag="g")
                        src_ap = tabsLO[l][:] if tbl == 0 else tabsHI[l][:]
                        nc.gpsimd.dma_gather(
                            out_ap=g[:, 0:w, :], in_ap=src_ap,
                            idxs_ap=itb[:, ic:ic + 8 * w],
                            num_idxs=P * w, num_idxs_reg=P * w, elem_size=ROW,
                            queue_num=next_q())
                        gts.append((g, w, col))
                    # scores: s = a_src + a_dst, leaky-relu, exp, cast
                    for (g, w, col) in gts:
                        gs = g[:, 0:w, HC:EXTB].bitcast(F32)   # [P, w, 8]
                        nc.vector.tensor_tensor(
                            out=sw[:, col:col + w, :], in0=gs[:, :, 0:H],
                            in1=adst[:, k, :].unsqueeze(1).to_broadcast([P, w, H]),
                            op=AX.add)
                    nc.vector.tensor_scalar(out=sb2[:, 0:Wt, :], in0=sw[:, 0:Wt, :],
                                            scalar1=NEG_SLOPE, scalar2=None, op0=AX.mult)
                    nc.vector.tensor_tensor(out=sw[:, 0:Wt, :], in0=sw[:, 0:Wt, :],
                                            in1=sb2[:, 0:Wt, :], op=AX.max)
                    nc.scalar.activation(swb[:, 0:Wt, :], sw[:, 0:Wt, :], AF.Exp)
                    # weighted messages
                    for (g, w, col) in gts:
                        nc.vector.tensor_tensor(
                            out=prod[:, col:col + w, :].rearrange("p w (h c) -> p w h c", h=H),
                            in0=g[:, 0:w, 0:HC].rearrange("p w (h c) -> p w h c", h=H),
                            in1=swb[:, col:col + w, :].unsqueeze(3).to_broadcast([P, w, H, C]),
                            op=AX.mult)
                    # fold prod over W -> num [P, HC] f32
                    num = spool.tile([P, HC], F32, name="num", tag="num")
                    lvl = Wt
                    while lvl > 2:
                        halfw = lvl // 2
                        nc.vector.tensor_tensor(out=prod[:, 0:halfw, :],
                                                in0=prod[:, 0:halfw, :],
                                                in1=prod[:, lvl - halfw:lvl, :],
                                                op=AX.add)
                        lvl -= halfw
                    if lvl == 2:
                        nc.vector.tensor_tensor(out=num[:], in0=prod[:, 0, :],
                                                in1=prod[:, 1, :], op=AX.add)
                    else:
                        nc.vector.tensor_copy(out=num[:], in_=prod[:, 0, :])
                    # denominator: fold swb over W -> dn [P, H] f32
                    dn = spool.tile([P, H], F32, name="dn", tag="dn")
                    lvl = Wt
                    while lvl > 2:
                        halfw = lvl // 2
                        nc.vector.tensor_tensor(out=swb[:, 0:halfw, :],
                                                in0=swb[:, 0:halfw, :],
                                                in1=swb[:, lvl - halfw:lvl, :],
                                                op=AX.add)
                        lvl -= halfw
                    if lvl == 2:
                        nc.vector.tensor_tensor(out=dn[:], in0=swb[:, 0, :],
                                                in1=swb[:, 1, :], op=AX.add)
                    else:
                        nc.vector.tensor_copy(out=dn[:], in_=swb[:, 0, :])
                    nc.vector.tensor_scalar(out=dn[:], in0=dn[:], scalar1=1e-30,
                                            scalar2=None, op0=AX.add)
                    rec = spool.tile([P, H], F32, name="rec", tag="rec")
                    nc.vector.reciprocal(rec[:], dn[:])
                    y = spool.tile([P, HC], F32, name="y", tag="y")
                    nc.vector.tensor_tensor(
                        out=y[:].rearrange("p (h c) -> p h c", h=H),
                        in0=num[:].rearrange("p (h c) -> p h c", h=H),
                        in1=rec[:].unsqueeze(2).to_broadcast([P, H, C]),
                        op=AX.mult)
                    nc.vector.tensor_tensor(out=y[:], in0=y[:], in1=bias[:, l, :], op=AX.add)
                    # ELU: y = max(y,0) + exp(min(y,0)) - 1
                    ey = spool.tile([P, HC], F32, name="ey", tag="ey")
                    nc.vector.tensor_scalar(out=ey[:], in0=y[:], scalar1=0.0,
                                            scalar2=None, op0=AX.min)
                    nc.scalar.activation(ey[:], ey[:], AF.Exp)
                    nc.vector.tensor_scalar(out=y[:], in0=y[:], scalar1=0.0,
                                            scalar2=-1.0, op0=AX.max, op1=AX.add)
                    nc.vector.tensor_tensor(out=y[:], in0=y[:], in1=ey[:], op=AX.add)
                    # transpose y
                    yt = spool.tile([P, HC], F32, name="yt", tag="yt")
                    for half in range(2):
                        ptt = pp2.tile([P, P], F32, name="pt", tag="pt")
                        nc.tensor.transpose(out=ptt[:], in_=y[:, half * P:(half + 1) * P],
                                            identity=iden[:])
                        nc.vector.tensor_copy(out=yt[:, half * P:(half + 1) * P], in_=ptt[:])
                    if l < 2:
                        we = w1e if l == 0 else w2e
                        ps = pp.tile([P, EXT], F32, name="psmm", tag="psmm")
                        nc.tensor.matmul(ps[:], lhsT=yt[:, 0:P], rhs=we[:, 0, :],
                                         start=True, stop=False)
                        nc.tensor.matmul(ps[:], lhsT=yt[:, P:HC], rhs=we[:, 1, :],
                                         start=False, stop=True)
                        write_hb(ps, k, bounce[l + 1][k * P:(k + 1) * P, 0:EXTB], l + 1)
                        if k == KBH - 1:
                            ag_half(l + 1, 0)
                    else:
                        zp = pp2.tile([P, C], F32, name="zp", tag="pt")
                        nc.tensor.matmul(zp[:], lhsT=yt[:, 0:P], rhs=hw1[:, 0, :],
                                         start=True, stop=False)
                        nc.tensor.matmul(zp[:], lhsT=yt[:, P:HC], rhs=hw1[:, 1, :],
                                         start=False, stop=True)
                        z = spool.tile([P, C], F32, name="z", tag="z")
                        nc.vector.tensor_tensor(out=z[:], in0=zp[:], in1=hb1[:], op=AX.add)
                        nc.scalar.activation(z[:], z[:], AF.Relu)
                        ztp = pp2.tile([P, P], F32, name="ztp", tag="pt")
                        nc.tensor.transpose(out=ztp[0:C, 0:P], in_=z[:, 0:C], identity=iden[:])
                        zt = spool.tile([C, P], F32, name="zt", tag="zt")
                        nc.vector.tensor_copy(out=zt[:], in_=ztp[0:C, 0:P])
                        op_ = pp2.tile([P, 1], F32, name="op_", tag="pt")
                        nc.tensor.matmul(op_[:], lhsT=zt[:], rhs=hw2[:], start=True, stop=True)
                        o = spool.tile([P, 1], F32, name="o", tag="o")
                        nc.vector.tensor_scalar(out=o[:], in0=op_[:], scalar1=hb2,
                                                scalar2=None, op0=AX.add)
                        nc.sync.dma_start(out_d[k * P:(k + 1) * P, :], o[:])
                if l < 2:
                    ag_half(l + 1, 1)

    nc.compile()
    return nc


def kernel(**inputs):
    x = np.asarray(inputs["x"], np.float32)
    edge_index = np.asarray(inputs["edge_index"])
    params = {k: np.asarray(v) for k, v in inputs.items() if k not in ("x", "edge_index")}

    geom, idx_bufs, xtt_bufs, smk_bufs, consts, core_nodes = _host_prep(x, edge_index, params)

    key = (geom["N"], geom["E"], geom["KB"], tuple(geom["Wlo"]), tuple(geom["Whi"]))
    if key not in _cache:
        _cache[key] = _build_program(geom, consts)
    nc = _cache[key]

    in_maps = []
    for c in range(NC):
        in_maps.append({
            "xtt": xtt_bufs[c],
            "idxbuf": idx_bufs[c],
            "srcmask": smk_bufs[c],
            "w0ext": consts["w0ext"],
            "w1ext": consts["w1ext"],
            "w2ext": consts["w2ext"],
            "bias": consts["bias"],
            "identity": consts["identity"],
            "hw1": consts["hw1"],
            "hb1": consts["hb1"],
            "hw2": consts["hw2"],
        })
    import os
    trace = os.environ.get("GAT_KERNEL_TRACE") == "1"
    res = bass_utils.run_bass_kernel_spmd(nc, in_maps, core_ids=list(range(NC)),
                                          trace=trace)
    kernel._last_exec_ns = res.exec_time_ns
    out = np.zeros(geom["N"], np.float32)
    for c in range(NC):
        blk = core_nodes[c]
        real = blk >= 0
        out[blk[real]] = res.results[c]["outv"][:, 0][real]
    return out
